# revision 31
# baseline (speedup 1.0000x reference)
"""Trainium2 Bass kernel: paged int8-KV-cache GQA decode attention, 8-core SPMD.

Contract: kernel(**inputs) takes the FULL unsharded numpy inputs (as produced by
the reference setup_inputs) and returns the FULL [32, 4096] float32 output.

Strategy (data parallel over sequence-chunks, flash-decoding style):
  - The 32 sequences' token tiles (ceil(ctx/128) each) are carved into
    8 cores x SLOTS contiguous chunks; slot s has a fixed tile count L[s]
    shared by all cores (SPMD), chosen by a small search to minimize padding
    (sum(L) ~ 6% over the ideal total/8).  A long sequence may span chunks on
    several cores; every chunk computes unnormalized partials (PV^T, Z) and
    the host combines: out = sum(PV) / sum(Z) (softmax without max-shift is
    linear in the partials).
  - Host staging is permutation/layout only: the int8-valued int32 KV cache is
    gathered per block_tables into per-core packed int8 buffers, slot-major so
    each (slot, 4-kvh-group) load is one contiguous DRAM block (128 big DMA
    descriptors).  The new decode token is quantized and scattered exactly as
    the reference's store_kvcache does, before the gather.
  - On device, SWDGE DMAs cast int8 -> bf16 inline during the HBM->SBUF load
    (the DMA engines charge max(src,dst) bytes, so this hits the bf16-landing
    floor of ~45 MB/core); k_scale * softmax_scale and v_scale are folded in
    as per-token vectors after the QK matmul / after exp respectively.
  - Per (slot, group of 4 kv heads):
      scores [128t, 4kvh, n, 4h] = per-tile matmuls(lhsT=K^T tile, rhs=q^T)
      s1 = scores * ksb  (DVE; ksb = k_scale*SCALE, zeroed beyond ctx)
      e  = exp(s1) in bf16 (ACT), one op per 4-kvh group
      ev = e * v_scale_vec (DVE; v_scale zeroed beyond ctx)
      Z  = matmul(lhsT=e, rhs=ones) per kvh -> fold to [1, 32]; pad tokens
           contribute exp(0)=1 each, corrected host-side via the known count
      PV = matmul(lhsT=V tile [128t,128d], rhs=ev [128t,4]) accumulated in
           PSUM as out^T [128d, 4h].
  Softmax skips max-subtraction (scores are O(20) at most; fp32 exp is safe).
"""

import os
import sys
import math
from contextlib import ExitStack

import numpy as np

sys.path.insert(0, "/opt/trn_rl_repo")

import ml_dtypes  # noqa: E402

import concourse.bass as bass  # noqa: E402
import concourse.mybir as mybir  # noqa: E402
import concourse.tile as tile  # noqa: E402
from concourse import bacc  # noqa: E402
from concourse.bass_utils import run_bass_kernel_spmd  # noqa: E402

BF16 = ml_dtypes.bfloat16

B = 32
NUM_HEADS = 32
KVH = 8
D = 128
REP = NUM_HEADS // KVH  # 4
BLOCK_SIZE = 256
T = 4096
P = 128
SCALE = 1.0 / float(np.sqrt(D))
NCORES = 8


# ---------------------------------------------------------------------------
# host-side planning + packing
# ---------------------------------------------------------------------------

def _greedy_chunks(tiles, L):
    """Slot-by-slot, give the 8 largest remaining sequences a chunk of up to
    L[s] tiles.  Returns per-slot lists of (seq, start_tile, len) or None if
    some sequence is left uncovered."""
    rem = [int(t) for t in tiles]
    start = [0] * len(tiles)
    chunks = []
    for Ls in L:
        order = sorted(range(len(tiles)), key=lambda b: -rem[b])
        sc = []
        for c in range(NCORES):
            b = order[c]
            ln = min(rem[b], Ls)
            sc.append((b, start[b], ln))
            rem[b] -= ln
            start[b] += ln
        chunks.append(sc)
    if any(r > 0 for r in rem):
        return None
    return chunks


_PLAN_CACHE = {}


def _plan(context_lens):
    """Choose slot lengths L and the (core, slot) -> sequence-chunk map."""
    tiles = tuple(int(math.ceil(int(c) / P)) for c in context_lens)
    if tiles in _PLAN_CACHE:
        return _PLAN_CACHE[tiles]
    ts = sorted(tiles, reverse=True)
    # octile fallback (always feasible): whole sequences, 4 slots
    best = (ts[0] + ts[8] + ts[16] + ts[24], (ts[0], ts[8], ts[16], ts[24]))
    for L0 in range(max(2, ts[0] - 8), ts[0] + 1):
        for L1 in range(max(2, ts[8] - 8), min(L0, ts[8] + 4) + 1):
            for L2 in range(max(2, ts[16] - 7), min(L1, ts[16] + 4) + 1):
                for L3 in range(max(2, ts[24] - 5), min(L2, ts[24] + 4) + 1):
                    base = L0 + L1 + L2 + L3
                    for L4 in range(2, min(L3, 10) + 1):
                        for L5 in (0, *range(2, L4 + 1)):
                            l6r = (0,) if L5 == 0 else (0, *range(2, L5 + 1))
                            for L6 in l6r:
                                L = tuple(x for x in
                                          (L0, L1, L2, L3, L4, L5, L6) if x)
                                N = sum(L)
                                if N >= best[0]:
                                    continue
                                if _greedy_chunks(tiles, L) is not None:
                                    best = (N, L)
    L = list(best[1])
    chunks = _greedy_chunks(tiles, L)
    _PLAN_CACHE[tiles] = (L, chunks)
    return L, chunks


def _quantize(x):
    absmax = np.abs(x).max(axis=-1)
    scale = np.where(absmax > 0.0, absmax / 127.0, 1.0).astype(np.float32)
    xq = np.clip(np.round(x / scale[..., None]), -127.0, 127.0).astype(np.int32)
    return xq, scale


def _pack_inputs(inputs, L, chunks):
    q = inputs["q"].reshape(B, NUM_HEADS, D).astype(np.float32)
    k = inputs["k"].reshape(B, KVH, D).astype(np.float32)
    v = inputs["v"].reshape(B, KVH, D).astype(np.float32)
    kc = np.ascontiguousarray(inputs["k_cache_q"].reshape(-1, KVH, D))
    vc = np.ascontiguousarray(inputs["v_cache_q"].reshape(-1, KVH, D))
    ks = np.ascontiguousarray(inputs["k_scale"].reshape(-1, KVH)).astype(np.float32)
    vs = np.ascontiguousarray(inputs["v_scale"].reshape(-1, KVH)).astype(np.float32)
    bt = inputs["block_tables"]
    ctx = inputs["context_lens"]
    sm = inputs["slot_mapping"]

    # store_kvcache_int8: quantize the new token and scatter into the cache
    kq, ksn = _quantize(k)
    vq, vsn = _quantize(v)
    kc = kc.copy(); vc = vc.copy(); ks = ks.copy(); vs = vs.copy()
    kc[sm] = kq; vc[sm] = vq; ks[sm] = ksn; vs[sm] = vsn

    SLOTS = len(L)
    NTT = sum(L)
    NT = NTT * P
    offs = np.concatenate([[0], np.cumsum(L)])

    in_maps = []
    padcnt = np.zeros((NCORES, SLOTS), dtype=np.float64)
    for c in range(NCORES):
        # K block per (slot, 4-kvh group): [d, j2, tokens]; V: [p, j2, tile, d]
        kt_c = np.zeros((1, KVH * D * NT), dtype=np.int8)
        vp_c = np.zeros((1, KVH * P * NTT * D), dtype=np.int8)
        ksb_c = np.zeros((P, KVH * NTT), dtype=np.float32)
        vsb_c = np.zeros((P, KVH * NTT), dtype=BF16)
        qt_c = np.zeros((P, SLOTS * 32), dtype=np.float32)
        for s in range(SLOTS):
            b, t0, ln = chunks[s][c]
            n = L[s]
            nt = n * P
            o = int(offs[s])
            nvalid = max(0, min(int(ctx[b]) - t0 * P, ln * P))
            padcnt[c, s] = nt - nvalid
            if ln > 0:
                flat = (bt[b][:, None] * BLOCK_SIZE
                        + np.arange(BLOCK_SIZE, dtype=np.int64)[None, :]
                        ).reshape(-1)[t0 * P: t0 * P + ln * P]
                kg = np.zeros((nt, KVH, D), dtype=np.int8)
                vg = np.zeros((nt, KVH, D), dtype=np.int8)
                kg[: ln * P] = kc[flat]
                vg[: ln * P] = vc[flat]
                scg = np.zeros((nt, KVH), dtype=np.float32)
                svg = np.zeros((nt, KVH), dtype=np.float32)
                valid = (np.arange(nt) < nvalid)
                scg[: ln * P] = ks[flat] * SCALE
                svg[: ln * P] = vs[flat]
                scg *= valid[:, None]
                svg *= valid[:, None]
                kjdt = kg.transpose(1, 2, 0)                      # [KVH, D, nt]
                vpjid = vg.reshape(n, P, KVH, D).transpose(1, 2, 0, 3)
                for g in range(KVH // 2):
                    ko = 8 * o * D * P + g * 2 * D * nt
                    kt_c[0, ko: ko + 2 * D * nt] = (
                        kjdt[2 * g: 2 * g + 2].transpose(1, 0, 2).reshape(-1))
                    vo = 8 * o * P * D + g * 2 * P * n * D
                    vp_c[0, vo: vo + 2 * P * n * D] = (
                        vpjid[:, 2 * g: 2 * g + 2].reshape(-1))

                def sprd(a, dt):
                    return a.reshape(n, P, KVH).transpose(1, 2, 0).reshape(
                        P, KVH * n).astype(dt)
                ksb_c[:, o * KVH: (o + n) * KVH] = sprd(scg, np.float32)
                vsb_c[:, o * KVH: (o + n) * KVH] = sprd(svg, BF16)
            qt_c[:, s * 32: (s + 1) * 32] = q[b].transpose(1, 0)  # [D, 32]
        sel = np.tile(np.eye(4, dtype=np.float32), (32, 1))       # [128, 4]
        in_maps.append(dict(kt=kt_c, vp=vp_c, ksb=ksb_c, vsb=vsb_c,
                            qt=qt_c, sel=sel))
    return in_maps, padcnt


# ---------------------------------------------------------------------------
# device program
# ---------------------------------------------------------------------------

def _build_program(L):
    SLOTS = len(L)
    NTT = sum(L)
    NT = NTT * P
    offs = [0]
    for n in L:
        offs.append(offs[-1] + n)
    f32 = mybir.dt.float32
    bf16 = mybir.dt.bfloat16
    i8 = mybir.dt.int8
    EXP = mybir.ActivationFunctionType.Exp

    nc = bacc.Bacc("TRN2", target_bir_lowering=False, debug=False,
                   num_devices=NCORES)

    kt_d = nc.dram_tensor("kt", [1, KVH * D * NT], i8, kind="ExternalInput").ap()
    vp_d = nc.dram_tensor("vp", [1, KVH * P * NTT * D], i8,
                          kind="ExternalInput").ap()
    ksb_d = nc.dram_tensor("ksb", [P, KVH * NTT], f32, kind="ExternalInput").ap()
    vsb_d = nc.dram_tensor("vsb", [P, KVH * NTT], bf16, kind="ExternalInput").ap()
    qt_d = nc.dram_tensor("qt", [P, SLOTS * 32], f32, kind="ExternalInput").ap()
    sel_d = nc.dram_tensor("sel", [P, 4], f32, kind="ExternalInput").ap()
    pv_d = nc.dram_tensor("pv", [SLOTS, P, 32], f32, kind="ExternalOutput").ap()
    z_d = nc.dram_tensor("z", [SLOTS, KVH, 4], f32,
                         kind="ExternalOutput").ap()

    with tile.TileContext(nc) as tc, ExitStack() as ctx:
        const = ctx.enter_context(tc.tile_pool(name="const", bufs=1))
        kt_pool = ctx.enter_context(tc.tile_pool(name="ktp", bufs=4))
        v_pool = ctx.enter_context(tc.tile_pool(name="vpp", bufs=4))
        sc_pool = ctx.enter_context(tc.tile_pool(name="scp", bufs=4))
        work = ctx.enter_context(tc.tile_pool(name="wrk", bufs=3))
        tail = ctx.enter_context(tc.tile_pool(name="tl", bufs=2))
        ps_qk = ctx.enter_context(tc.tile_pool(name="psqk", bufs=3, space="PSUM"))
        ps_pt = ctx.enter_context(tc.tile_pool(name="pspt", bufs=2, space="PSUM"))
        ps_z = ctx.enter_context(tc.tile_pool(name="psz", bufs=1, space="PSUM"))
        ps_pv = ctx.enter_context(tc.tile_pool(name="pspv", bufs=2, space="PSUM"))

        qt_f = const.tile([P, SLOTS * 32], f32)
        nc.sync.dma_start(qt_f, qt_d)
        qt = const.tile([P, SLOTS * 32], bf16)
        nc.vector.tensor_copy(qt, qt_f)
        sel = const.tile([P, 4], f32)
        nc.sync.dma_start(sel, sel_d)
        ones = const.tile([P, 1], bf16)
        nc.vector.memset(ones, 1.0)

        # Slots >= RES_START are small; their K/V are loaded ONCE into
        # persistent tiles, enqueued right after slot 0's loads.  Their
        # latency-bound compute is interleaved BETWEEN the big streaming
        # slots so the kernel tail is a streaming slot, not a serial chain.
        RES_START = 3 if SLOTS > 4 else SLOTS
        res_tiles = {}
        order = list(range(SLOTS))
        pend = []

        for si, s in enumerate(order):
            n = L[s]
            o = offs[s]
            if s == RES_START:
                for r in range(RES_START, SLOTS):
                    nr = L[r]
                    orr = offs[r]
                    kr = const.tile([P, 4, 2, nr, P], bf16, tag=f"kr{r}")
                    ko = 8 * orr * D * P
                    nc.gpsimd.dma_start(
                        kr, kt_d[0:1, ko: ko + 8 * D * nr * P].rearrange(
                            "o (g d r) -> (o d) g r", g=4, d=P))
                    vr = const.tile([P, 4, 2, nr, D], bf16, tag=f"vr{r}")
                    vo = 8 * orr * P * D
                    nc.gpsimd.dma_start(
                        vr, vp_d[0:1, vo: vo + 8 * P * nr * D].rearrange(
                            "o (g p r) -> (o p) g r", g=4, p=P))
                    res_tiles[r] = (kr, vr)
            ksb_s = sc_pool.tile([P, KVH, n, 1], f32, tag="ksb")
            nc.sync.dma_start(ksb_s, ksb_d[:, o * KVH: (o + n) * KVH])
            vsb_s = sc_pool.tile([P, KVH, n, 1], bf16, tag="vsb")
            nc.sync.dma_start(vsb_s, vsb_d[:, o * KVH: (o + n) * KVH])

            pv = ps_pv.tile([P, 32], f32, tag="pv")
            pt = ps_pt.tile([P, KVH], f32, tag="pt")
            z_all = ps_z.tile([KVH, 4], f32, tag="z")

            # resident (small) slots batch all 8 kv heads into one chunk to
            # minimize cross-engine dependency chains; streaming slots use
            # 2-kvh chunks for pipelining.
            # PV/pt emission is deferred one chunk so the in-order PE stream
            # never head-blocks on the softmax chain: QK(c+1) runs while
            # DVE/ACT produce ev(c), then PV(c) is ready when reached.
            G = KVH if s >= RES_START else 2
            for jh in range(KVH // G):
                if s >= RES_START:
                    def kslc(j, i, s=s):
                        return res_tiles[s][0][:, j // 2, j % 2, i, :]
                    def vslc(j, i, s=s):
                        return res_tiles[s][1][:, j // 2, j % 2, i, :]
                else:
                    ktc = kt_pool.tile([P, 2, n, P], bf16, tag="kt")
                    vtc = v_pool.tile([P, 2, n, D], bf16, tag="vt")
                    ko = 8 * o * D * P + jh * 2 * D * n * P
                    nc.gpsimd.dma_start(
                        ktc,
                        kt_d[0:1, ko: ko + 2 * D * n * P].rearrange(
                            "o (d r) -> (o d) r", d=P))
                    vo = 8 * o * P * D + jh * 2 * P * n * D
                    nc.gpsimd.dma_start(
                        vtc,
                        vp_d[0:1, vo: vo + 2 * P * n * D].rearrange(
                            "o (p r) -> (o p) r", p=P))

                    def kslc(j, i, ktc=ktc):
                        return ktc[:, j % 2, i, :]
                    def vslc(j, i, vtc=vtc):
                        return vtc[:, j % 2, i, :]

                qk = ps_qk.tile([P, G, n, 4], f32, tag="qk")
                for j2 in range(G):
                    j = G * jh + j2
                    qcol = s * 32 + 4 * j
                    for i in range(n):
                        nc.tensor.matmul(
                            qk[:, j2, i, :],
                            lhsT=kslc(j, i),
                            rhs=qt[:, qcol: qcol + 4],
                            start=True, stop=True, skip_group_check=True)
                for f in pend:
                    f()
                pend.clear()

                s1 = work.tile([P, G, n, 4], f32, tag="s1")
                nc.vector.tensor_mul(
                    s1, qk,
                    ksb_s[:, G * jh: G * jh + G].to_broadcast([P, G, n, 4]))
                e = work.tile([P, G, n, 4], bf16, tag="e")
                nc.scalar.activation(e, s1, EXP)
                ev = work.tile([P, G, n, 4], bf16, tag="ev")
                nc.vector.tensor_mul(
                    ev, e,
                    vsb_s[:, G * jh: G * jh + G].to_broadcast([P, G, n, 4]))

                def emit_pv(e=e, ev=ev, jh=jh, G=G, n=n,
                            kslc=kslc, vslc=vslc):
                    for j2 in range(G):
                        j = G * jh + j2
                        # Z partials: per-(tile, head) column sums of e
                        nc.tensor.matmul(
                            pt[0: n * 4, j: j + 1],
                            lhsT=e[:, j2], rhs=ones,
                            start=True, stop=True, skip_group_check=True)
                        # PV accumulate over token tiles: out^T [128d, 4h]
                        cc = 4 * j
                        for i in range(n):
                            nc.tensor.matmul(
                                pv[:, cc: cc + 4],
                                lhsT=vslc(j, i),
                                rhs=ev[:, j2, i, :],
                                start=(i == 0), stop=(i == n - 1),
                                skip_group_check=True)
                pend.append(emit_pv)

            def epilogue(s=s, n=n, pt=pt, pv=pv, z_all=z_all):
                # fold Z partials -> [8kvh, 4h] in one matmul; the host
                # consumes z either way
                pts = tail.tile([P, KVH], f32, tag="pts")
                nc.vector.tensor_copy(pts[0: n * 4, :], pt[0: n * 4, :])
                nc.tensor.matmul(z_all, lhsT=pts[0: n * 4, :],
                                 rhs=sel[0: n * 4, :], start=True, stop=True)
                zs = tail.tile([KVH, 4], f32, tag="zs")
                nc.vector.tensor_copy(zs, z_all)
                nc.scalar.dma_start(z_d[s], zs)
                pvs = tail.tile([P, 32], f32, tag="pvs")
                nc.vector.tensor_copy(pvs, pv)
                nc.scalar.dma_start(pv_d[s], pvs)
            pend.append(epilogue)
        for f in pend:
            f()
        pend.clear()

    nc.compile()
    return nc


_PROGRAM_CACHE = {}


def _get_program(L):
    key = tuple(L)
    if key not in _PROGRAM_CACHE:
        _PROGRAM_CACHE[key] = _build_program(L)
    return _PROGRAM_CACHE[key]


# ---------------------------------------------------------------------------
# entry point
# ---------------------------------------------------------------------------

def kernel(q, k, v, k_cache_q, v_cache_q, k_scale, v_scale,
           block_tables, context_lens, slot_mapping, _trace=False):
    inputs = dict(q=np.asarray(q), k=np.asarray(k), v=np.asarray(v),
                  k_cache_q=np.asarray(k_cache_q),
                  v_cache_q=np.asarray(v_cache_q),
                  k_scale=np.asarray(k_scale), v_scale=np.asarray(v_scale),
                  block_tables=np.asarray(block_tables),
                  context_lens=np.asarray(context_lens),
                  slot_mapping=np.asarray(slot_mapping))
    L, chunks = _plan(inputs["context_lens"])
    in_maps, padcnt = _pack_inputs(inputs, L, chunks)
    nc = _get_program(L)
    res = run_bass_kernel_spmd(nc, in_maps, core_ids=list(range(NCORES)),
                               trace=_trace)

    # combine unnormalized partials across chunks (flash-decoding merge)
    accp = np.zeros((B, P, 32), dtype=np.float64)
    accz = np.zeros((B, 32), dtype=np.float64)
    for c in range(NCORES):
        pvs = res.results[c]["pv"]   # [SLOTS, P, 32]
        zss = res.results[c]["z"]    # [SLOTS, KVH, 4]
        for s in range(len(L)):
            b, _, _ = chunks[s][c]
            accp[b] += pvs[s]
            accz[b] += zss[s].reshape(32) - padcnt[c, s]
    out = (accp / accz[:, None, :]).transpose(0, 2, 1)  # [B, 32h, 128d]
    out = np.ascontiguousarray(out.reshape(B, NUM_HEADS * D), dtype=np.float32)
    if _trace:
        return out, res
    return out


# revision 33
# speedup vs baseline: 1.0514x; 1.0514x over previous
"""Trainium2 Bass kernel: paged int8-KV-cache GQA decode attention, 8-core SPMD.

Contract: kernel(**inputs) takes the FULL unsharded numpy inputs (as produced by
the reference setup_inputs) and returns the FULL [32, 4096] float32 output.

Strategy (data parallel over sequence-chunks, flash-decoding style):
  - The 32 sequences' token tiles (ceil(ctx/128) each) are carved into
    8 cores x SLOTS contiguous chunks; slot s has a fixed tile count L[s]
    shared by all cores (SPMD), chosen by a small search to minimize padding
    (sum(L) ~ 6% over the ideal total/8).  A long sequence may span chunks on
    several cores; every chunk computes unnormalized partials (PV^T, Z) and
    the host combines: out = sum(PV) / sum(Z) (softmax without max-shift is
    linear in the partials).
  - Host staging is permutation/layout only: the int8-valued int32 KV cache is
    gathered per block_tables into per-core packed int8 buffers, slot-major so
    each (slot, 4-kvh-group) load is one contiguous DRAM block (128 big DMA
    descriptors).  The new decode token is quantized and scattered exactly as
    the reference's store_kvcache does, before the gather.
  - On device, SWDGE DMAs cast int8 -> bf16 inline during the HBM->SBUF load
    (the DMA engines charge max(src,dst) bytes, so this hits the bf16-landing
    floor of ~45 MB/core); k_scale * softmax_scale and v_scale are folded in
    as per-token vectors after the QK matmul / after exp respectively.
  - Per (slot, group of 4 kv heads):
      scores [128t, 4kvh, n, 4h] = per-tile matmuls(lhsT=K^T tile, rhs=q^T)
      s1 = scores * ksb  (DVE; ksb = k_scale*SCALE, zeroed beyond ctx)
      e  = exp(s1) in bf16 (ACT), one op per 4-kvh group
      ev = e * v_scale_vec (DVE; v_scale zeroed beyond ctx)
      Z  = matmul(lhsT=e, rhs=ones) per kvh -> fold to [1, 32]; pad tokens
           contribute exp(0)=1 each, corrected host-side via the known count
      PV = matmul(lhsT=V tile [128t,128d], rhs=ev [128t,4]) accumulated in
           PSUM as out^T [128d, 4h].
  Softmax skips max-subtraction (scores are O(20) at most; fp32 exp is safe).
"""

import os
import sys
import math
from contextlib import ExitStack

import numpy as np

sys.path.insert(0, "/opt/trn_rl_repo")

import ml_dtypes  # noqa: E402

import concourse.bass as bass  # noqa: E402
import concourse.mybir as mybir  # noqa: E402
import concourse.tile as tile  # noqa: E402
from concourse import bacc  # noqa: E402
from concourse.bass_utils import run_bass_kernel_spmd  # noqa: E402

BF16 = ml_dtypes.bfloat16

B = 32
NUM_HEADS = 32
KVH = 8
D = 128
REP = NUM_HEADS // KVH  # 4
BLOCK_SIZE = 256
T = 4096
P = 128
SCALE = 1.0 / float(np.sqrt(D))
NCORES = 8


# ---------------------------------------------------------------------------
# host-side planning + packing
# ---------------------------------------------------------------------------

def _greedy_chunks(tiles, L):
    """Slot-by-slot, give the 8 largest remaining sequences a chunk of up to
    L[s] tiles.  Returns per-slot lists of (seq, start_tile, len) or None if
    some sequence is left uncovered."""
    rem = [int(t) for t in tiles]
    start = [0] * len(tiles)
    chunks = []
    for Ls in L:
        order = sorted(range(len(tiles)), key=lambda b: -rem[b])
        sc = []
        for c in range(NCORES):
            b = order[c]
            ln = min(rem[b], Ls)
            sc.append((b, start[b], ln))
            rem[b] -= ln
            start[b] += ln
        chunks.append(sc)
    if any(r > 0 for r in rem):
        return None
    return chunks


_PLAN_CACHE = {}


def _plan(context_lens):
    """Choose slot lengths L and the (core, slot) -> sequence-chunk map."""
    tiles = tuple(int(math.ceil(int(c) / P)) for c in context_lens)
    if tiles in _PLAN_CACHE:
        return _PLAN_CACHE[tiles]
    ts = sorted(tiles, reverse=True)
    # octile fallback (always feasible): whole sequences, 4 slots
    best = (ts[0] + ts[8] + ts[16] + ts[24], (ts[0], ts[8], ts[16], ts[24]))
    for L0 in range(max(2, ts[0] - 8), ts[0] + 1):
        for L1 in range(max(2, ts[8] - 8), min(L0, ts[8] + 4) + 1):
            for L2 in range(max(2, ts[16] - 7), min(L1, ts[16] + 4) + 1):
                for L3 in range(max(2, ts[24] - 5), min(L2, ts[24] + 4) + 1):
                    base = L0 + L1 + L2 + L3
                    for L4 in range(2, min(L3, 10) + 1):
                        for L5 in (0, *range(2, L4 + 1)):
                            l6r = (0,) if L5 == 0 else (0, *range(2, L5 + 1))
                            for L6 in l6r:
                                L = tuple(x for x in
                                          (L0, L1, L2, L3, L4, L5, L6) if x)
                                N = sum(L)
                                if N >= best[0]:
                                    continue
                                if _greedy_chunks(tiles, L) is not None:
                                    best = (N, L)
    L = list(best[1])
    chunks = _greedy_chunks(tiles, L)
    _PLAN_CACHE[tiles] = (L, chunks)
    return L, chunks


def _quantize(x):
    absmax = np.abs(x).max(axis=-1)
    scale = np.where(absmax > 0.0, absmax / 127.0, 1.0).astype(np.float32)
    xq = np.clip(np.round(x / scale[..., None]), -127.0, 127.0).astype(np.int32)
    return xq, scale


def _pack_inputs(inputs, L, chunks):
    q = inputs["q"].reshape(B, NUM_HEADS, D).astype(np.float32)
    k = inputs["k"].reshape(B, KVH, D).astype(np.float32)
    v = inputs["v"].reshape(B, KVH, D).astype(np.float32)
    kc = np.ascontiguousarray(inputs["k_cache_q"].reshape(-1, KVH, D))
    vc = np.ascontiguousarray(inputs["v_cache_q"].reshape(-1, KVH, D))
    ks = np.ascontiguousarray(inputs["k_scale"].reshape(-1, KVH)).astype(np.float32)
    vs = np.ascontiguousarray(inputs["v_scale"].reshape(-1, KVH)).astype(np.float32)
    bt = inputs["block_tables"]
    ctx = inputs["context_lens"]
    sm = inputs["slot_mapping"]

    # store_kvcache_int8: quantize the new token and scatter into the cache
    kq, ksn = _quantize(k)
    vq, vsn = _quantize(v)
    kc = kc.copy(); vc = vc.copy(); ks = ks.copy(); vs = vs.copy()
    kc[sm] = kq; vc[sm] = vq; ks[sm] = ksn; vs[sm] = vsn

    SLOTS = len(L)
    NTT = sum(L)
    NT = NTT * P
    offs = np.concatenate([[0], np.cumsum(L)])

    in_maps = []
    padcnt = np.zeros((NCORES, SLOTS), dtype=np.float64)
    for c in range(NCORES):
        # K block per (slot, 4-kvh group): [d, j2, tokens]; V: [p, j2, tile, d]
        kt_c = np.zeros((1, KVH * D * NT), dtype=np.int8)
        vp_c = np.zeros((1, KVH * P * NTT * D), dtype=np.int8)
        ksb_c = np.zeros((P, KVH * NTT), dtype=np.float32)
        vsb_c = np.zeros((P, KVH * NTT), dtype=BF16)
        qt_c = np.zeros((P, SLOTS * 32), dtype=np.float32)
        for s in range(SLOTS):
            b, t0, ln = chunks[s][c]
            n = L[s]
            nt = n * P
            o = int(offs[s])
            nvalid = max(0, min(int(ctx[b]) - t0 * P, ln * P))
            padcnt[c, s] = nt - nvalid
            if ln > 0:
                flat = (bt[b][:, None] * BLOCK_SIZE
                        + np.arange(BLOCK_SIZE, dtype=np.int64)[None, :]
                        ).reshape(-1)[t0 * P: t0 * P + ln * P]
                kg = np.zeros((nt, KVH, D), dtype=np.int8)
                vg = np.zeros((nt, KVH, D), dtype=np.int8)
                kg[: ln * P] = kc[flat]
                vg[: ln * P] = vc[flat]
                scg = np.zeros((nt, KVH), dtype=np.float32)
                svg = np.zeros((nt, KVH), dtype=np.float32)
                valid = (np.arange(nt) < nvalid)
                scg[: ln * P] = ks[flat] * SCALE
                svg[: ln * P] = vs[flat]
                scg *= valid[:, None]
                svg *= valid[:, None]
                kjdt = kg.transpose(1, 2, 0)                      # [KVH, D, nt]
                vpjid = vg.reshape(n, P, KVH, D).transpose(1, 2, 0, 3)
                for g in range(KVH // 2):
                    ko = 8 * o * D * P + g * 2 * D * nt
                    kt_c[0, ko: ko + 2 * D * nt] = (
                        kjdt[2 * g: 2 * g + 2].transpose(1, 0, 2).reshape(-1))
                    vo = 8 * o * P * D + g * 2 * P * n * D
                    vp_c[0, vo: vo + 2 * P * n * D] = (
                        vpjid[:, 2 * g: 2 * g + 2].reshape(-1))

                def sprd(a, dt):
                    return a.reshape(n, P, KVH).transpose(1, 2, 0).reshape(
                        P, KVH * n).astype(dt)
                ksb_c[:, o * KVH: (o + n) * KVH] = sprd(scg, np.float32)
                vsb_c[:, o * KVH: (o + n) * KVH] = sprd(svg, BF16)
            qt_c[:, s * 32: (s + 1) * 32] = q[b].transpose(1, 0)  # [D, 32]
        sel = np.tile(np.eye(4, dtype=np.float32), (32, 1))       # [128, 4]
        in_maps.append(dict(kt=kt_c, vp=vp_c, ksb=ksb_c, vsb=vsb_c,
                            qt=qt_c, sel=sel))
    return in_maps, padcnt


# ---------------------------------------------------------------------------
# device program
# ---------------------------------------------------------------------------

def _build_program(L):
    SLOTS = len(L)
    NTT = sum(L)
    NT = NTT * P
    offs = [0]
    for n in L:
        offs.append(offs[-1] + n)
    f32 = mybir.dt.float32
    bf16 = mybir.dt.bfloat16
    i8 = mybir.dt.int8
    EXP = mybir.ActivationFunctionType.Exp

    nc = bacc.Bacc("TRN2", target_bir_lowering=False, debug=False,
                   num_devices=NCORES)

    kt_d = nc.dram_tensor("kt", [1, KVH * D * NT], i8, kind="ExternalInput").ap()
    vp_d = nc.dram_tensor("vp", [1, KVH * P * NTT * D], i8,
                          kind="ExternalInput").ap()
    ksb_d = nc.dram_tensor("ksb", [P, KVH * NTT], f32, kind="ExternalInput").ap()
    vsb_d = nc.dram_tensor("vsb", [P, KVH * NTT], bf16, kind="ExternalInput").ap()
    qt_d = nc.dram_tensor("qt", [P, SLOTS * 32], f32, kind="ExternalInput").ap()
    sel_d = nc.dram_tensor("sel", [P, 4], f32, kind="ExternalInput").ap()
    pv_d = nc.dram_tensor("pv", [SLOTS, P, 32], f32, kind="ExternalOutput").ap()
    z_d = nc.dram_tensor("z", [SLOTS, KVH, 4], f32,
                         kind="ExternalOutput").ap()

    with tile.TileContext(nc) as tc, ExitStack() as ctx:
        const = ctx.enter_context(tc.tile_pool(name="const", bufs=1))
        kt_pool = ctx.enter_context(tc.tile_pool(name="ktp", bufs=4))
        v_pool = ctx.enter_context(tc.tile_pool(name="vpp", bufs=4))
        sc_pool = ctx.enter_context(tc.tile_pool(name="scp", bufs=4))
        work = ctx.enter_context(tc.tile_pool(name="wrk", bufs=3))
        tail = ctx.enter_context(tc.tile_pool(name="tl", bufs=2))
        ps_qk = ctx.enter_context(tc.tile_pool(name="psqk", bufs=3, space="PSUM"))
        ps_pt = ctx.enter_context(tc.tile_pool(name="pspt", bufs=2, space="PSUM"))
        ps_z = ctx.enter_context(tc.tile_pool(name="psz", bufs=1, space="PSUM"))
        ps_pv = ctx.enter_context(tc.tile_pool(name="pspv", bufs=2, space="PSUM"))

        qt_f = const.tile([P, SLOTS * 32], f32)
        nc.sync.dma_start(qt_f, qt_d)
        qt = const.tile([P, SLOTS * 32], bf16)
        nc.vector.tensor_copy(qt, qt_f)
        sel = const.tile([P, 4], f32)
        nc.sync.dma_start(sel, sel_d)
        ones = const.tile([P, 1], bf16)
        nc.vector.memset(ones, 1.0)

        # Slots >= RES_START are small; their K/V are loaded ONCE into
        # persistent tiles, enqueued right after slot 0's loads.  Their
        # latency-bound compute is interleaved BETWEEN the big streaming
        # slots so the kernel tail is a streaming slot, not a serial chain.
        RES_START = 3 if SLOTS > 4 else SLOTS
        res_tiles = {}
        if RES_START < SLOTS:
            order = [0] + list(range(RES_START, SLOTS)) + [1, 2]
        else:
            order = list(range(SLOTS))

        for si, s in enumerate(order):
            n = L[s]
            o = offs[s]
            if si == 1 and RES_START < SLOTS:
                for r in range(RES_START, SLOTS):
                    nr = L[r]
                    orr = offs[r]
                    kr = const.tile([P, 4, 2, nr, P], bf16, tag=f"kr{r}")
                    ko = 8 * orr * D * P
                    nc.gpsimd.dma_start(
                        kr, kt_d[0:1, ko: ko + 8 * D * nr * P].rearrange(
                            "o (g d r) -> (o d) g r", g=4, d=P))
                    vr = const.tile([P, 4, 2, nr, D], bf16, tag=f"vr{r}")
                    vo = 8 * orr * P * D
                    nc.gpsimd.dma_start(
                        vr, vp_d[0:1, vo: vo + 8 * P * nr * D].rearrange(
                            "o (g p r) -> (o p) g r", g=4, p=P))
                    res_tiles[r] = (kr, vr)
            ksb_s = sc_pool.tile([P, KVH, n, 1], f32, tag="ksb")
            nc.sync.dma_start(ksb_s, ksb_d[:, o * KVH: (o + n) * KVH])
            vsb_s = sc_pool.tile([P, KVH, n, 1], bf16, tag="vsb")
            nc.sync.dma_start(vsb_s, vsb_d[:, o * KVH: (o + n) * KVH])

            pv = ps_pv.tile([P, 32], f32, tag="pv")
            pt = ps_pt.tile([P, KVH], f32, tag="pt")
            z_all = ps_z.tile([KVH, 4], f32, tag="z")

            # resident (small) slots batch all 8 kv heads into one chunk to
            # minimize cross-engine dependency chains; streaming slots use
            # 2-kvh chunks for pipelining.
            G = KVH if s >= RES_START else 2
            pend = [None]
            for jh in range(KVH // G):
                if s >= RES_START:
                    def kslc(j, i, s=s):
                        return res_tiles[s][0][:, j // 2, j % 2, i, :]
                    def vslc(j, i, s=s):
                        return res_tiles[s][1][:, j // 2, j % 2, i, :]
                else:
                    ktc = kt_pool.tile([P, 2, n, P], bf16, tag="kt")
                    vtc = v_pool.tile([P, 2, n, D], bf16, tag="vt")
                    ko = 8 * o * D * P + jh * 2 * D * n * P
                    nc.gpsimd.dma_start(
                        ktc,
                        kt_d[0:1, ko: ko + 2 * D * n * P].rearrange(
                            "o (d r) -> (o d) r", d=P))
                    vo = 8 * o * P * D + jh * 2 * P * n * D
                    nc.gpsimd.dma_start(
                        vtc,
                        vp_d[0:1, vo: vo + 2 * P * n * D].rearrange(
                            "o (p r) -> (o p) r", p=P))

                    def kslc(j, i, ktc=ktc):
                        return ktc[:, j % 2, i, :]
                    def vslc(j, i, vtc=vtc):
                        return vtc[:, j % 2, i, :]

                qk = ps_qk.tile([P, G, n, 4], f32, tag="qk")
                for j2 in range(G):
                    j = G * jh + j2
                    qcol = s * 32 + 4 * j
                    for i in range(n):
                        nc.tensor.matmul(
                            qk[:, j2, i, :],
                            lhsT=kslc(j, i),
                            rhs=qt[:, qcol: qcol + 4],
                            start=True, stop=True, skip_group_check=True)
                if pend[0] is not None:
                    pend[0]()

                s1 = work.tile([P, G, n, 4], f32, tag="s1")
                nc.vector.tensor_mul(
                    s1, qk,
                    ksb_s[:, G * jh: G * jh + G].to_broadcast([P, G, n, 4]))
                e = work.tile([P, G, n, 4], bf16, tag="e")
                nc.scalar.activation(e, s1, EXP)
                ev = work.tile([P, G, n, 4], bf16, tag="ev")
                nc.vector.tensor_mul(
                    ev, e,
                    vsb_s[:, G * jh: G * jh + G].to_broadcast([P, G, n, 4]))

                def emit_pv(e=e, ev=ev, jh=jh, G=G, n=n, vslc=vslc):
                    for j2 in range(G):
                        j = G * jh + j2
                        # Z partials: per-(tile, head) column sums of e
                        nc.tensor.matmul(
                            pt[0: n * 4, j: j + 1],
                            lhsT=e[:, j2], rhs=ones,
                            start=True, stop=True, skip_group_check=True)
                        # PV accumulate over token tiles: out^T [128d, 4h]
                        cc = 4 * j
                        for i in range(n):
                            nc.tensor.matmul(
                                pv[:, cc: cc + 4],
                                lhsT=vslc(j, i),
                                rhs=ev[:, j2, i, :],
                                start=(i == 0), stop=(i == n - 1),
                                skip_group_check=True)
                pend[0] = emit_pv
            pend[0]()

            # fold Z partials -> [8kvh, 4h] in one matmul; host reads either way
            pts = tail.tile([P, KVH], f32, tag="pts")
            nc.vector.tensor_copy(pts[0: n * 4, :], pt[0: n * 4, :])
            nc.tensor.matmul(z_all, lhsT=pts[0: n * 4, :],
                             rhs=sel[0: n * 4, :], start=True, stop=True)
            zs = tail.tile([KVH, 4], f32, tag="zs")
            nc.vector.tensor_copy(zs, z_all)
            nc.scalar.dma_start(z_d[s], zs)
            pvs = tail.tile([P, 32], f32, tag="pvs")
            nc.vector.tensor_copy(pvs, pv)
            nc.scalar.dma_start(pv_d[s], pvs)

    nc.compile()
    return nc


_PROGRAM_CACHE = {}


def _get_program(L):
    key = tuple(L)
    if key not in _PROGRAM_CACHE:
        _PROGRAM_CACHE[key] = _build_program(L)
    return _PROGRAM_CACHE[key]


# ---------------------------------------------------------------------------
# entry point
# ---------------------------------------------------------------------------

def kernel(q, k, v, k_cache_q, v_cache_q, k_scale, v_scale,
           block_tables, context_lens, slot_mapping, _trace=False):
    inputs = dict(q=np.asarray(q), k=np.asarray(k), v=np.asarray(v),
                  k_cache_q=np.asarray(k_cache_q),
                  v_cache_q=np.asarray(v_cache_q),
                  k_scale=np.asarray(k_scale), v_scale=np.asarray(v_scale),
                  block_tables=np.asarray(block_tables),
                  context_lens=np.asarray(context_lens),
                  slot_mapping=np.asarray(slot_mapping))
    L, chunks = _plan(inputs["context_lens"])
    in_maps, padcnt = _pack_inputs(inputs, L, chunks)
    nc = _get_program(L)
    res = run_bass_kernel_spmd(nc, in_maps, core_ids=list(range(NCORES)),
                               trace=_trace)

    # combine unnormalized partials across chunks (flash-decoding merge)
    accp = np.zeros((B, P, 32), dtype=np.float64)
    accz = np.zeros((B, 32), dtype=np.float64)
    for c in range(NCORES):
        pvs = res.results[c]["pv"]   # [SLOTS, P, 32]
        zss = res.results[c]["z"]    # [SLOTS, KVH, 4]
        for s in range(len(L)):
            b, _, _ = chunks[s][c]
            accp[b] += pvs[s]
            accz[b] += zss[s].reshape(32) - padcnt[c, s]
    out = (accp / accz[:, None, :]).transpose(0, 2, 1)  # [B, 32h, 128d]
    out = np.ascontiguousarray(out.reshape(B, NUM_HEADS * D), dtype=np.float32)
    if _trace:
        return out, res
    return out
